# revision 28
# baseline (speedup 1.0000x reference)
"""GNN NodeBlock kernel for 8 TRN2 NeuronCores.

Strategy:
  - Host: sort edges by receiver, partition receivers into 8 contiguous
    node ranges (6250 nodes/core).  Each core owns its node slice
    end-to-end -> the [N,h] aggregate never needs an all-reduce.
  - Device per core: segment-sum via one-hot matmul.  Edges are grouped
    into blocks of SPAN=126 receiver nodes; a per-block [128, T*128]
    one-hot is built in ONE wide compare op (iota vs broadcast per-edge
    local index, is_equal), alternating between the Vector and GpSimd
    engines; PE accumulates  psum[feat, span] += edge_tile^T @ onehot.
    One-hot col 127 is a real-edge indicator, so psum col 127 yields the
    block's edge-feature sum (feeds the global mean) for free.
    The aggregate is produced TRANSPOSED [h, nodes], matching the lhsT
    layout the MLP matmuls want, so no on-device transposes anywhere.
  - EDGE_MODE "f16": edge features stream as fp16 (half the HBM
    traffic; end-to-end output error ~2e-4 RMS, accumulation still f32
    in PSUM).  EDGE_MODE "exact": bf16 hi+lo split per edge value -- the
    same double-pass the HW uses for fp32 matmuls (~2e-6 RMS).
  - Mean subtraction (ib_e) folds into the first-layer bias:
    b1' = b1 - (sum_agg/N) @ W1b  -> only a [128,1] AllReduce (with an
    early dummy AllReduce to absorb the ~75us cold-start of the CC
    engine).  First-layer MLP matmuls don't depend on it and are
    prestaged into SBUF while the collective is in flight.
  - MLP: h_T = relu(W1a^T @ nodeT + W1b^T @ aggT + b1'),
    x_T = W2^T @ h_T + b2, streamed out per 500-node chunk with the
    elementwise stages alternating between Vector and Scalar engines.
  - Host: concat per-core x_T slices, transpose, return.
"""

import numpy as np
import ml_dtypes

N_NODES = 50000
N_EDGES = 800000
H = 128
N_CORES = 8
NPC = N_NODES // N_CORES          # nodes per core
SPAN = 126                        # receiver-node block width (even, <=128)
NBLK = (NPC + SPAN - 1) // SPAN   # blocks per core (must be even)
MLP_CHUNK = 500                   # MLP free-dim chunk (<=512 psum bank)
OHW = 128                         # one-hot group width (126 iota + pad + indicator)
EDGE_MODE = "f16"                 # "f16" (fast) or "exact" (bf16 hi/lo)

_cache = {}


def _build_program(T, mode):
    import concourse.bacc as bacc
    import concourse.mybir as mybir
    import concourse.tile as tile

    f32 = mybir.dt.float32
    edt = mybir.dt.float16 if mode == "f16" else mybir.dt.bfloat16
    npass = 1 if mode == "f16" else 2
    nc = bacc.Bacc("TRN2", target_bir_lowering=False, debug=False,
                   num_devices=N_CORES)

    # edge slabs, two blocks (f16) or hi+lo of one block (exact) glued
    # per partition row -> 8704B DMA rows
    ecb = nc.dram_tensor("ecb", [NBLK // (3 - npass), 128, 2 * T * 128], edt,
                         kind="ExternalInput")
    recv = nc.dram_tensor("recv", [128, NBLK * T], edt, kind="ExternalInput")
    nodeT = nc.dram_tensor("nodeT", [128, NPC], f32, kind="ExternalInput")
    w1a = nc.dram_tensor("w1a", [128, 128], f32, kind="ExternalInput")
    w1b = nc.dram_tensor("w1b", [128, 128], f32, kind="ExternalInput")
    w2 = nc.dram_tensor("w2", [128, 128], f32, kind="ExternalInput")
    b1c = nc.dram_tensor("b1c", [128, 1], f32, kind="ExternalInput")
    b2r = nc.dram_tensor("b2r", [1, 128], f32, kind="ExternalInput")
    iot = nc.dram_tensor("iot", [128, T * SPAN], edt, kind="ExternalInput")
    out = nc.dram_tensor("out", [128, NPC], f32, kind="ExternalOutput")

    with tile.TileContext(nc) as tc:
        with (
            tc.tile_pool(name="const", bufs=1) as cpool,
            tc.tile_pool(name="edges", bufs=4) as epool,
            tc.tile_pool(name="oh", bufs=4) as ohpool,
            tc.tile_pool(name="mlp", bufs=3) as mpool,
            tc.tile_pool(name="pagg", bufs=2, space="PSUM") as pagg,
            tc.tile_pool(name="pmlp", bufs=2, space="PSUM") as pmlp,
            tc.tile_pool(name="dram", bufs=1, space="DRAM") as dpool,
        ):
            # warm up the CC engine so the real AllReduce later is not a
            # ~75us cold start (its result is never read)
            ccw_in = dpool.tile([128, 1], f32, tag="ccw_in")
            ccw_out = dpool.tile([128, 1], f32, tag="ccw_out")
            nc.gpsimd.collective_compute(
                "AllReduce", mybir.AluOpType.add,
                replica_groups=[list(range(N_CORES))],
                ins=[ccw_in.opt()], outs=[ccw_out.opt()])

            # iota/recv head the scalar queue (small, needed by the first
            # one-hot); edge slabs own the sync queue from t=0; the big
            # nodeT load sits behind iota/recv on the scalar queue since
            # nothing needs it until the MLP prestage.
            iota_sb = cpool.tile([128, T * SPAN], edt)
            nc.scalar.dma_start(iota_sb[:], iot[:])
            recv_sb = cpool.tile([128, NBLK * T], edt)
            nc.scalar.dma_start(recv_sb[:], recv[:])
            w1a_sb = cpool.tile([128, 128], f32)
            nc.scalar.dma_start(w1a_sb[:], w1a[:])
            w1b_sb = cpool.tile([128, 128], f32)
            nc.scalar.dma_start(w1b_sb[:], w1b[:])
            w2_sb = cpool.tile([128, 128], f32)
            nc.scalar.dma_start(w2_sb[:], w2[:])
            b1_sb = cpool.tile([128, 1], f32)
            nc.scalar.dma_start(b1_sb[:], b1c[:])
            b2r_sb = cpool.tile([1, 128], f32)
            nc.scalar.dma_start(b2r_sb[:], b2r[:])
            ones_sb = cpool.tile([1, MLP_CHUNK], f32)
            nc.vector.memset(ones_sb[:], 1.0)
            nodeT_sb = cpool.tile([128, NPC], f32)
            nc.scalar.dma_start(nodeT_sb[:], nodeT[:])
            agg_sb = cpool.tile([128, NBLK * SPAN], f32)
            sums_sb = cpool.tile([128, NBLK], f32)
            hpre_sb = cpool.tile([128, NPC], f32)
            # real-edge indicator for ALL tiles in one DVE op; per-block
            # strided copies into one-hot col 127 run on the scalar engine
            ind_sb = cpool.tile([128, NBLK * T], edt)
            nc.vector.tensor_scalar(ind_sb[:], recv_sb[:], 0.0, None,
                                    op0=mybir.AluOpType.is_ge)

            # ---- phase 1: segment-sum into agg_sb (transposed [h, node]) ----
            def do_block(b, hi, lo):
                oh = ohpool.tile([128, T * OHW], edt)
                ohv = oh[:].rearrange("p (t s) -> p t s", s=OHW)
                rc = recv_sb[:, b * T:(b + 1) * T]
                nc.vector.tensor_tensor(
                    ohv[:, :, 0:SPAN],
                    iota_sb[:].rearrange("p (t s) -> p t s", s=SPAN),
                    rc.unsqueeze(2).broadcast_to([128, T, SPAN]),
                    op=mybir.AluOpType.is_equal)
                nc.scalar.copy(ohv[:, :, 127:128],
                               ind_sb[:, b * T:(b + 1) * T].unsqueeze(2))
                pa = pagg.tile([128, OHW], f32)
                for t in range(T):
                    ts = slice(t * 128, (t + 1) * 128)
                    ss = slice(t * OHW, (t + 1) * OHW)
                    if lo is None:
                        nc.tensor.matmul(pa[:], hi[:, ts], oh[:, ss],
                                         start=(t == 0), stop=(t == T - 1))
                    else:
                        nc.tensor.matmul(pa[:], hi[:, ts], oh[:, ss],
                                         start=(t == 0), stop=False)
                        nc.tensor.matmul(pa[:], lo[:, ts], oh[:, ss],
                                         start=False, stop=(t == T - 1))
                nc.scalar.copy(agg_sb[:, b * SPAN:(b + 1) * SPAN], pa[:, 0:SPAN])
                nc.scalar.copy(sums_sb[:, b:b + 1], pa[:, 127:128])

            if mode == "f16":
                for bp in range(NBLK // 2):
                    eb = epool.tile([128, 2 * T * 128], edt, tag="eb")
                    nc.sync.dma_start(eb[:], ecb[bp])
                    do_block(2 * bp, eb[:, 0:T * 128], None)
                    do_block(2 * bp + 1, eb[:, T * 128:2 * T * 128], None)
            else:
                for b in range(NBLK):
                    eb = epool.tile([128, 2 * T * 128], edt, tag="eb")
                    nc.sync.dma_start(eb[:], ecb[b])
                    do_block(b, eb[:, 0:T * 128], eb[:, T * 128:2 * T * 128])

            # ---- phase 2: column-mean correction via tiny AllReduce ----
            # all-reduce the raw per-block sums (25KB) straight from the
            # scalar engine that wrote them -- no pre-CC reduce hop; the
            # [128,NBLK] reduction folds into the mu matmul + one DVE
            # reduce afterwards.
            cc_in = dpool.tile([128, NBLK], f32)
            cc_out = dpool.tile([128, NBLK], f32)
            nc.scalar.dma_start(cc_in[:], sums_sb[:])
            nc.gpsimd.collective_compute(
                "AllReduce", mybir.AluOpType.add,
                replica_groups=[list(range(N_CORES))],
                ins=[cc_in.opt()], outs=[cc_out.opt()])
            mu_sb = cpool.tile([128, NBLK], f32)
            nc.scalar.dma_start(mu_sb[:], cc_out[:])
            ps_mu = pmlp.tile([128, NBLK], f32, tag="psmu", bufs=1)
            nc.tensor.matmul(ps_mu[:], w1b_sb[:], mu_sb[:], start=True, stop=True)
            mu_red = cpool.tile([128, 1], f32)
            nc.vector.reduce_sum(mu_red[:], ps_mu[:], axis=mybir.AxisListType.X)
            b1_adj = cpool.tile([128, 1], f32)
            nc.vector.scalar_tensor_tensor(
                b1_adj[:], mu_red[:], -1.0 / N_NODES, b1_sb[:],
                op0=mybir.AluOpType.mult, op1=mybir.AluOpType.add)

            # ---- phase 3a: prestage first-layer matmuls (overlap the cc) ----
            chunks = [slice(s, min(s + MLP_CHUNK, NPC))
                      for s in range(0, NPC, MLP_CHUNK)]
            for sl in chunks:
                w = sl.stop - sl.start
                ph = pmlp.tile([128, MLP_CHUNK], f32, tag="ph")
                nc.tensor.matmul(ph[:, 0:w], w1a_sb[:], nodeT_sb[:, sl],
                                 start=True, stop=False)
                nc.tensor.matmul(ph[:, 0:w], w1b_sb[:], agg_sb[:, sl],
                                 start=False, stop=True)
                nc.scalar.copy(hpre_sb[:, sl], ph[:, 0:w])

            # ---- phase 3b: finish MLP once b1_adj is known ----
            # relu+bias / output-bias alternate between DVE and ACT so the
            # two lanes pipeline in parallel
            for ci, sl in enumerate(chunks):
                w = sl.stop - sl.start
                h = mpool.tile([128, MLP_CHUNK], f32, tag="h")
                if ci % 2 == 0:
                    nc.vector.tensor_scalar(h[:, 0:w], hpre_sb[:, sl],
                                            b1_adj[:, 0:1], 0.0,
                                            op0=mybir.AluOpType.add,
                                            op1=mybir.AluOpType.max)
                else:
                    nc.scalar.activation(h[:, 0:w], hpre_sb[:, sl],
                                         mybir.ActivationFunctionType.Relu,
                                         bias=b1_adj[:, 0:1])
                px = pmlp.tile([128, MLP_CHUNK], f32, tag="px", bufs=3)
                nc.tensor.matmul(px[:, 0:w], b2r_sb[:], ones_sb[:, 0:w],
                                 start=True, stop=False)
                nc.tensor.matmul(px[:, 0:w], w2_sb[:], h[:, 0:w],
                                 start=False, stop=True)
                xo = mpool.tile([128, MLP_CHUNK], f32, tag="xo")
                if ci % 2 == 0:
                    nc.vector.tensor_copy(xo[:, 0:w], px[:, 0:w])
                else:
                    nc.scalar.copy(xo[:, 0:w], px[:, 0:w])
                nc.sync.dma_start(out[:, sl], xo[:, 0:w])

    nc.compile()
    return nc


def _prepare(node_attr, edge_index, edge_attr, W1, b1, W2, b2):
    node_attr = np.ascontiguousarray(np.asarray(node_attr, np.float32))
    edge_attr = np.ascontiguousarray(np.asarray(edge_attr, np.float32))
    W1 = np.asarray(W1, np.float32)
    b1 = np.asarray(b1, np.float32)
    W2 = np.ascontiguousarray(np.asarray(W2, np.float32))
    b2 = np.asarray(b2, np.float32)
    receivers = np.asarray(edge_index)[1]

    perm = np.argsort(receivers, kind="stable")
    srecv = receivers[perm]

    # block edge ranges: (core, blk) -> receiver range [lo_node, hi_node)
    lows = []
    for c in range(N_CORES):
        for b in range(NBLK):
            lows.append(min(c * NPC + b * SPAN, (c + 1) * NPC))
    lows.append(N_NODES)
    bounds = np.searchsorted(srecv, np.array(lows), side="left")
    cnts = np.diff(bounds).reshape(N_CORES, NBLK)
    T = max(1, int(np.max((cnts + 127) // 128)))

    edt = np.float16 if EDGE_MODE == "f16" else ml_dtypes.bfloat16
    nodeT = np.ascontiguousarray(node_attr.T)          # [128, N]
    w1a = np.ascontiguousarray(W1[:H])
    w1b = np.ascontiguousarray(W1[H:])
    iota = np.ascontiguousarray(np.broadcast_to(
        np.tile(np.arange(SPAN, dtype=np.float32), T), (128, T * SPAN))).astype(edt)

    in_maps = []
    k = 0
    for c in range(N_CORES):
        epad = np.zeros((NBLK, T * 128, H), np.float32)
        rpad = np.full((NBLK, T * 128), -1.0, np.float32)
        for b in range(NBLK):
            lo, hi = bounds[k], bounds[k + 1]
            cnt = hi - lo
            if cnt:
                epad[b, :cnt] = edge_attr[perm[lo:hi]]
                rpad[b, :cnt] = (srecv[lo:hi] - lows[k]).astype(np.float32)
            k += 1
        # slot s = t*128 + p -> DRAM layout [blk, p, t, f] / recv [p, blk*T+t]
        epad = np.ascontiguousarray(
            epad.reshape(NBLK, T, 128, H).transpose(0, 2, 1, 3)
        ).reshape(NBLK, 128, T * 128)
        if EDGE_MODE == "f16":
            e16 = epad.astype(np.float16)
            # glue two consecutive blocks per slab row
            ecb = np.ascontiguousarray(
                e16.reshape(NBLK // 2, 2, 128, T * 128).transpose(0, 2, 1, 3)
            ).reshape(NBLK // 2, 128, 2 * T * 128)
        else:
            ehi = epad.astype(edt)
            elo = (epad - ehi.astype(np.float32)).astype(edt)
            ecb = np.ascontiguousarray(np.concatenate([ehi, elo], axis=2))
        rpad = np.ascontiguousarray(
            rpad.reshape(NBLK, T, 128).transpose(2, 0, 1)
        ).reshape(128, NBLK * T).astype(edt)
        in_maps.append({
            "ecb": ecb, "recv": rpad,
            "nodeT": np.ascontiguousarray(nodeT[:, c * NPC:(c + 1) * NPC]),
            "w1a": w1a, "w1b": w1b, "w2": W2,
            "b1c": np.ascontiguousarray(b1.reshape(H, 1)),
            "b2r": np.ascontiguousarray(b2.reshape(1, H)),
            "iot": iota,
        })
    return T, in_maps


def _run(inputs, trace=False):
    from concourse.bass_utils import run_bass_kernel_spmd

    T, in_maps = _prepare(**inputs)
    key = (T, EDGE_MODE)
    if _cache.get("key") != key:
        _cache["nc"] = _build_program(T, EDGE_MODE)
        _cache["key"] = key
    res = run_bass_kernel_spmd(_cache["nc"], in_maps,
                               core_ids=list(range(N_CORES)), trace=trace)
    xT = np.concatenate([res.results[c]["out"] for c in range(N_CORES)], axis=1)
    x = np.ascontiguousarray(xT.T)
    return x, res


def kernel(node_attr, edge_index, edge_attr, W1, b1, W2, b2):
    inputs = dict(node_attr=node_attr, edge_index=edge_index,
                  edge_attr=edge_attr, W1=W1, b1=b1, W2=W2, b2=b2)
    x, _ = _run(inputs, trace=False)
    return (x, np.asarray(edge_index), np.asarray(edge_attr))


# revision 29
# speedup vs baseline: 1.1989x; 1.1989x over previous
"""GNN NodeBlock kernel for 8 TRN2 NeuronCores.

Strategy:
  - Host: sort edges by receiver, partition receivers into 8 contiguous
    node ranges (6250 nodes/core).  Each core owns its node slice
    end-to-end -> the [N,h] aggregate never needs an all-reduce.
  - Device per core: segment-sum via one-hot matmul.  Edges are grouped
    into blocks of SPAN=126 receiver nodes; a per-block [128, T*128]
    one-hot is built in ONE wide compare op (iota vs broadcast per-edge
    local index, is_equal), alternating between the Vector and GpSimd
    engines; PE accumulates  psum[feat, span] += edge_tile^T @ onehot.
    One-hot col 127 is a real-edge indicator, so psum col 127 yields the
    block's edge-feature sum (feeds the global mean) for free.
    The aggregate is produced TRANSPOSED [h, nodes], matching the lhsT
    layout the MLP matmuls want, so no on-device transposes anywhere.
  - EDGE_MODE "f16": edge features stream as fp16 (half the HBM
    traffic; end-to-end output error ~2e-4 RMS, accumulation still f32
    in PSUM).  EDGE_MODE "exact": bf16 hi+lo split per edge value -- the
    same double-pass the HW uses for fp32 matmuls (~2e-6 RMS).
  - Mean subtraction (ib_e) folds into the first-layer bias:
    b1' = b1 - (sum_agg/N) @ W1b  -> only a [128,1] AllReduce (with an
    early dummy AllReduce to absorb the ~75us cold-start of the CC
    engine).  First-layer MLP matmuls don't depend on it and are
    prestaged into SBUF while the collective is in flight.
  - MLP: h_T = relu(W1a^T @ nodeT + W1b^T @ aggT + b1'),
    x_T = W2^T @ h_T + b2, streamed out per 500-node chunk with the
    elementwise stages alternating between Vector and Scalar engines.
  - Host: concat per-core x_T slices, transpose, return.
"""

import numpy as np
import ml_dtypes

N_NODES = 50000
N_EDGES = 800000
H = 128
N_CORES = 8
NPC = N_NODES // N_CORES          # nodes per core
SPAN = 126                        # receiver-node block width (even, <=128)
NBLK = (NPC + SPAN - 1) // SPAN   # blocks per core (must be even)
MLP_CHUNK = 500                   # MLP free-dim chunk (<=512 psum bank)
OHW = 128                         # one-hot group width (126 iota + pad + indicator)
EDGE_MODE = "f16"                 # "f16" (fast) or "exact" (bf16 hi/lo)

_cache = {}


def _build_program(T, mode):
    import concourse.bacc as bacc
    import concourse.mybir as mybir
    import concourse.tile as tile

    f32 = mybir.dt.float32
    edt = mybir.dt.float16 if mode == "f16" else mybir.dt.bfloat16
    npass = 1 if mode == "f16" else 2
    nc = bacc.Bacc("TRN2", target_bir_lowering=False, debug=False,
                   num_devices=N_CORES)

    # edge slabs, two blocks (f16) or hi+lo of one block (exact) glued
    # per partition row -> 8704B DMA rows
    ecb = nc.dram_tensor("ecb", [NBLK // (3 - npass), 128, 2 * T * 128], edt,
                         kind="ExternalInput")
    recv = nc.dram_tensor("recv", [128, NBLK * T], edt, kind="ExternalInput")
    nodeT = nc.dram_tensor("nodeT", [128, NPC], f32, kind="ExternalInput")
    w1a = nc.dram_tensor("w1a", [128, 128], f32, kind="ExternalInput")
    w1b = nc.dram_tensor("w1b", [128, 128], f32, kind="ExternalInput")
    w2 = nc.dram_tensor("w2", [128, 128], f32, kind="ExternalInput")
    b1c = nc.dram_tensor("b1c", [128, 1], f32, kind="ExternalInput")
    b2c = nc.dram_tensor("b2c", [128, 1], f32, kind="ExternalInput")
    iot = nc.dram_tensor("iot", [128, T * SPAN], edt, kind="ExternalInput")
    out = nc.dram_tensor("out", [128, NPC], f32, kind="ExternalOutput")

    with tile.TileContext(nc) as tc:
        with (
            tc.tile_pool(name="const", bufs=1) as cpool,
            tc.tile_pool(name="edges", bufs=4) as epool,
            tc.tile_pool(name="oh", bufs=4) as ohpool,
            tc.tile_pool(name="mlp", bufs=3) as mpool,
            tc.tile_pool(name="pagg", bufs=2, space="PSUM") as pagg,
            tc.tile_pool(name="pmlp", bufs=2, space="PSUM") as pmlp,
            tc.tile_pool(name="dram", bufs=1, space="DRAM") as dpool,
        ):
            # warm up the CC engine so the real AllReduce later is not a
            # ~75us cold start (its result is never read)
            ccw_in = dpool.tile([128, 1], f32, tag="ccw_in")
            ccw_out = dpool.tile([128, 1], f32, tag="ccw_out")
            nc.gpsimd.collective_compute(
                "AllReduce", mybir.AluOpType.add,
                replica_groups=[list(range(N_CORES))],
                ins=[ccw_in.opt()], outs=[ccw_out.opt()])

            # iota/recv head the scalar queue (small, needed by the first
            # one-hot); edge slabs own the sync queue from t=0; the big
            # nodeT load sits behind iota/recv on the scalar queue since
            # nothing needs it until the MLP prestage.
            iota_sb = cpool.tile([128, T * SPAN], edt)
            nc.scalar.dma_start(iota_sb[:], iot[:])
            recv_sb = cpool.tile([128, NBLK * T], edt)
            nc.scalar.dma_start(recv_sb[:], recv[:])
            w1a_sb = cpool.tile([128, 128], f32)
            nc.scalar.dma_start(w1a_sb[:], w1a[:])
            w1b_sb = cpool.tile([128, 128], f32)
            nc.scalar.dma_start(w1b_sb[:], w1b[:])
            w2_sb = cpool.tile([128, 128], f32)
            nc.scalar.dma_start(w2_sb[:], w2[:])
            b1_sb = cpool.tile([128, 1], f32)
            nc.scalar.dma_start(b1_sb[:], b1c[:])
            b2_sb = cpool.tile([128, 1], f32)
            nc.scalar.dma_start(b2_sb[:], b2c[:])
            nodeT_sb = cpool.tile([128, NPC], f32)
            nc.scalar.dma_start(nodeT_sb[:], nodeT[:])
            agg_sb = cpool.tile([128, NBLK * SPAN], f32)
            sums_sb = cpool.tile([128, NBLK], f32)
            hpre_sb = cpool.tile([128, NPC], f32)
            # real-edge indicator for ALL tiles in one DVE op; per-block
            # strided copies into one-hot col 127 run on the scalar engine
            ind_sb = cpool.tile([128, NBLK * T], edt)
            nc.vector.tensor_scalar(ind_sb[:], recv_sb[:], 0.0, None,
                                    op0=mybir.AluOpType.is_ge)

            # ---- phase 1: segment-sum into agg_sb (transposed [h, node]) ----
            def do_block(b, hi, lo):
                oh = ohpool.tile([128, T * OHW], edt)
                ohv = oh[:].rearrange("p (t s) -> p t s", s=OHW)
                rc = recv_sb[:, b * T:(b + 1) * T]
                nc.vector.tensor_tensor(
                    ohv[:, :, 0:SPAN],
                    iota_sb[:].rearrange("p (t s) -> p t s", s=SPAN),
                    rc.unsqueeze(2).broadcast_to([128, T, SPAN]),
                    op=mybir.AluOpType.is_equal)
                nc.scalar.copy(ohv[:, :, 127:128],
                               ind_sb[:, b * T:(b + 1) * T].unsqueeze(2))
                pa = pagg.tile([128, OHW], f32)
                for t in range(T):
                    ts = slice(t * 128, (t + 1) * 128)
                    ss = slice(t * OHW, (t + 1) * OHW)
                    if lo is None:
                        nc.tensor.matmul(pa[:], hi[:, ts], oh[:, ss],
                                         start=(t == 0), stop=(t == T - 1))
                    else:
                        nc.tensor.matmul(pa[:], hi[:, ts], oh[:, ss],
                                         start=(t == 0), stop=False)
                        nc.tensor.matmul(pa[:], lo[:, ts], oh[:, ss],
                                         start=False, stop=(t == T - 1))
                nc.scalar.copy(agg_sb[:, b * SPAN:(b + 1) * SPAN], pa[:, 0:SPAN])
                nc.scalar.copy(sums_sb[:, b:b + 1], pa[:, 127:128])

            if mode == "f16":
                for bp in range(NBLK // 2):
                    eb = epool.tile([128, 2 * T * 128], edt, tag="eb")
                    nc.sync.dma_start(eb[:], ecb[bp])
                    do_block(2 * bp, eb[:, 0:T * 128], None)
                    do_block(2 * bp + 1, eb[:, T * 128:2 * T * 128], None)
            else:
                for b in range(NBLK):
                    eb = epool.tile([128, 2 * T * 128], edt, tag="eb")
                    nc.sync.dma_start(eb[:], ecb[b])
                    do_block(b, eb[:, 0:T * 128], eb[:, T * 128:2 * T * 128])

            # ---- phase 2: column-mean correction via tiny AllReduce ----
            # all-reduce the raw per-block sums (25KB) straight from the
            # scalar engine that wrote them -- no pre-CC reduce hop; the
            # [128,NBLK] reduction folds into the mu matmul + one DVE
            # reduce afterwards.
            cc_in = dpool.tile([128, NBLK], f32)
            cc_out = dpool.tile([128, NBLK], f32)
            nc.scalar.dma_start(cc_in[:], sums_sb[:])
            nc.gpsimd.collective_compute(
                "AllReduce", mybir.AluOpType.add,
                replica_groups=[list(range(N_CORES))],
                ins=[cc_in.opt()], outs=[cc_out.opt()])
            mu_sb = cpool.tile([128, NBLK], f32)
            nc.scalar.dma_start(mu_sb[:], cc_out[:])
            ps_mu = pmlp.tile([128, NBLK], f32, tag="psmu", bufs=1)
            nc.tensor.matmul(ps_mu[:], w1b_sb[:], mu_sb[:], start=True, stop=True)
            mu_red = cpool.tile([128, 1], f32)
            nc.vector.reduce_sum(mu_red[:], ps_mu[:], axis=mybir.AxisListType.X)
            b1_adj = cpool.tile([128, 1], f32)
            nc.vector.scalar_tensor_tensor(
                b1_adj[:], mu_red[:], -1.0 / N_NODES, b1_sb[:],
                op0=mybir.AluOpType.mult, op1=mybir.AluOpType.add)

            # ---- phase 3a: prestage first-layer matmuls (overlap the cc) ----
            chunks = [slice(s, min(s + MLP_CHUNK, NPC))
                      for s in range(0, NPC, MLP_CHUNK)]
            for sl in chunks:
                w = sl.stop - sl.start
                ph = pmlp.tile([128, MLP_CHUNK], f32, tag="ph")
                nc.tensor.matmul(ph[:, 0:w], w1a_sb[:], nodeT_sb[:, sl],
                                 start=True, stop=False)
                nc.tensor.matmul(ph[:, 0:w], w1b_sb[:], agg_sb[:, sl],
                                 start=False, stop=True)
                nc.scalar.copy(hpre_sb[:, sl], ph[:, 0:w])

            # ---- phase 3b: finish MLP once b1_adj is known ----
            # relu+bias / output-bias alternate between DVE and ACT so the
            # two lanes pipeline in parallel
            for ci, sl in enumerate(chunks):
                w = sl.stop - sl.start
                h = mpool.tile([128, MLP_CHUNK], f32, tag="h")
                if ci % 2 == 0:
                    nc.vector.tensor_scalar(h[:, 0:w], hpre_sb[:, sl],
                                            b1_adj[:, 0:1], 0.0,
                                            op0=mybir.AluOpType.add,
                                            op1=mybir.AluOpType.max)
                else:
                    nc.scalar.activation(h[:, 0:w], hpre_sb[:, sl],
                                         mybir.ActivationFunctionType.Relu,
                                         bias=b1_adj[:, 0:1])
                px = pmlp.tile([128, MLP_CHUNK], f32, tag="px", bufs=3)
                nc.tensor.matmul(px[:, 0:w], w2_sb[:], h[:, 0:w],
                                 start=True, stop=True)
                xo = mpool.tile([128, MLP_CHUNK], f32, tag="xo")
                if ci % 2 == 0:
                    nc.scalar.activation(xo[:, 0:w], px[:, 0:w],
                                         mybir.ActivationFunctionType.Identity,
                                         bias=b2_sb[:, 0:1])
                else:
                    nc.vector.tensor_scalar(xo[:, 0:w], px[:, 0:w],
                                            b2_sb[:, 0:1], None,
                                            op0=mybir.AluOpType.add)
                nc.sync.dma_start(out[:, sl], xo[:, 0:w])

    nc.compile()
    return nc


def _prepare(node_attr, edge_index, edge_attr, W1, b1, W2, b2):
    node_attr = np.ascontiguousarray(np.asarray(node_attr, np.float32))
    edge_attr = np.ascontiguousarray(np.asarray(edge_attr, np.float32))
    W1 = np.asarray(W1, np.float32)
    b1 = np.asarray(b1, np.float32)
    W2 = np.ascontiguousarray(np.asarray(W2, np.float32))
    b2 = np.asarray(b2, np.float32)
    receivers = np.asarray(edge_index)[1]

    perm = np.argsort(receivers, kind="stable")
    srecv = receivers[perm]

    # block edge ranges: (core, blk) -> receiver range [lo_node, hi_node)
    lows = []
    for c in range(N_CORES):
        for b in range(NBLK):
            lows.append(min(c * NPC + b * SPAN, (c + 1) * NPC))
    lows.append(N_NODES)
    bounds = np.searchsorted(srecv, np.array(lows), side="left")
    cnts = np.diff(bounds).reshape(N_CORES, NBLK)
    T = max(1, int(np.max((cnts + 127) // 128)))

    edt = np.float16 if EDGE_MODE == "f16" else ml_dtypes.bfloat16
    nodeT = np.ascontiguousarray(node_attr.T)          # [128, N]
    w1a = np.ascontiguousarray(W1[:H])
    w1b = np.ascontiguousarray(W1[H:])
    iota = np.ascontiguousarray(np.broadcast_to(
        np.tile(np.arange(SPAN, dtype=np.float32), T), (128, T * SPAN))).astype(edt)

    in_maps = []
    k = 0
    for c in range(N_CORES):
        epad = np.zeros((NBLK, T * 128, H), np.float32)
        rpad = np.full((NBLK, T * 128), -1.0, np.float32)
        for b in range(NBLK):
            lo, hi = bounds[k], bounds[k + 1]
            cnt = hi - lo
            if cnt:
                epad[b, :cnt] = edge_attr[perm[lo:hi]]
                rpad[b, :cnt] = (srecv[lo:hi] - lows[k]).astype(np.float32)
            k += 1
        # slot s = t*128 + p -> DRAM layout [blk, p, t, f] / recv [p, blk*T+t]
        epad = np.ascontiguousarray(
            epad.reshape(NBLK, T, 128, H).transpose(0, 2, 1, 3)
        ).reshape(NBLK, 128, T * 128)
        if EDGE_MODE == "f16":
            e16 = epad.astype(np.float16)
            # glue two consecutive blocks per slab row
            ecb = np.ascontiguousarray(
                e16.reshape(NBLK // 2, 2, 128, T * 128).transpose(0, 2, 1, 3)
            ).reshape(NBLK // 2, 128, 2 * T * 128)
        else:
            ehi = epad.astype(edt)
            elo = (epad - ehi.astype(np.float32)).astype(edt)
            ecb = np.ascontiguousarray(np.concatenate([ehi, elo], axis=2))
        rpad = np.ascontiguousarray(
            rpad.reshape(NBLK, T, 128).transpose(2, 0, 1)
        ).reshape(128, NBLK * T).astype(edt)
        in_maps.append({
            "ecb": ecb, "recv": rpad,
            "nodeT": np.ascontiguousarray(nodeT[:, c * NPC:(c + 1) * NPC]),
            "w1a": w1a, "w1b": w1b, "w2": W2,
            "b1c": np.ascontiguousarray(b1.reshape(H, 1)),
            "b2c": np.ascontiguousarray(b2.reshape(H, 1)),
            "iot": iota,
        })
    return T, in_maps


def _run(inputs, trace=False):
    from concourse.bass_utils import run_bass_kernel_spmd

    T, in_maps = _prepare(**inputs)
    key = (T, EDGE_MODE)
    if _cache.get("key") != key:
        _cache["nc"] = _build_program(T, EDGE_MODE)
        _cache["key"] = key
    res = run_bass_kernel_spmd(_cache["nc"], in_maps,
                               core_ids=list(range(N_CORES)), trace=trace)
    xT = np.concatenate([res.results[c]["out"] for c in range(N_CORES)], axis=1)
    x = np.ascontiguousarray(xT.T)
    return x, res


def kernel(node_attr, edge_index, edge_attr, W1, b1, W2, b2):
    inputs = dict(node_attr=node_attr, edge_index=edge_index,
                  edge_attr=edge_attr, W1=W1, b1=b1, W2=W2, b2=b2)
    x, _ = _run(inputs, trace=False)
    return (x, np.asarray(edge_index), np.asarray(edge_attr))
